# revision 42
# baseline (speedup 1.0000x reference)
"""Trainium2 Bass kernel for nn_Attention_57423712748130.

Computation (per batch b):
  X4 = x[b] viewed (C=256, N=4096)   [raw reshape]
  Q4 = silu(BN(q_w @ X4))            (256, 4096)
  KV4 = silu(BN(kv_w @ Y4))          (128, 4096)
  q[n,h,d]  = Q4[n1, n0*256+h*64+d]      n = n1*16+n0
  k[m,h,d]  = KV4[m1, m0*512 + h*64+d]   m = m1*8+m0
  v[m,h,d]  = KV4[m1, m0*512+256+h*64+d]
  att = softmax(q k^T / 8); o = att v
  out rows [h*1024,(h+1)*1024) = O_h @ proj_w.T + proj_b
    where O_h[n2, n3*64+d] = o[4*n2+n3, d]

Sharding: 8 cores = (batch b in 0..3) x (head-pair hp in 0..1); each core
computes heads {2hp, 2hp+1} of batch b = rows [hp*2048,(hp+1)*2048) of out[b].

On-core strategy (v2 — ScalarE(exp)-bound design, ~all else overlapped):
 - all bulk inputs/weights arrive as bf16 host-prefolded to [128, 2W]
   (row halves side by side) and load as 2D piece-DMAs spread over the 3
   DMA queues (sync/scalar/gpsimd; ~2.6us serial latency per DMA/queue,
   ~230 GB/s shared port); output returns bf16, upcast on host
 - conv outputs are computed directly in transposed layout so q^T/k^T
   need no transposes; conv bias is a trailing K=1 matmul (ones (x)
   bias_row) in the same PSUM accumulation; silu computed as
   z*(1+tanh(z/2)) = 2*silu(z) (tanh shares the ACT table set with exp,
   preloaded via a dummy op at t=0); the 2x is folded into the exp scale
   (1/32) and the 2.0 ones-column of V
 - a ~3.4us warmup burst of full-array matmuls on the first-landed
   weight flips the HAM clock gate to 8/8 (2.4 GHz) before real work
 - scoresT[m,n]: ONE [128,1024] psum per m0-block holds BOTH heads side
   by side, so the two score matmuls (PE row groups 0-63/64-127) become
   ready together and overlap in the array; one [128,1024] exp each;
   scores run one t2 AHEAD of exp so ScalarE (the bottleneck: 64 exp +
   12 tanh ops) never waits; av interleaves both heads by m0, chasing
   exp pairwise
 - att@v contracts over m with an extra 2.0-column on V producing
   softmax denominators as psum row 64; 1/denom = one DVE copy to SBUF
   (reciprocal_approx_fast reads garbage from PSUM on HW) + one
   reciprocal_approx_fast, gpsimd partition_broadcast per 256-col half,
   multiply folded into the PSUM-evacuation copy — no DRAM bounce
 - normalization + per-head projection run per q0-quarter, pipelined
   behind the attention of later quarters; O_h columns kept q0-major so
   every PSUM evacuation is contiguous; the output DMA un-permutes rows
"""

import ml_dtypes
import numpy as np

B = 4
N_TOK = 4096
C = 256
BN_EPS = 1e-5

_CACHE = {}


def _build():
    import concourse.bacc as bacc
    import concourse.bass as bass
    import concourse.tile as tile
    from concourse import mybir

    f32 = mybir.dt.float32
    bf16 = mybir.dt.bfloat16
    adt = bf16
    AF = mybir.ActivationFunctionType

    nc = bacc.Bacc("TRN2", target_bir_lowering=False, debug=False, num_devices=8)

    # all [256,W] operands arrive host-prefolded to [128, 2W] (row half
    # c0 side by side) so each loads with ONE 2D DMA; bf16 bias rows are
    # packed into one tensor (per-DMA queue latency is ~2.5-3us, serial)
    xq = nc.dram_tensor("xq", [128, 4096], bf16, kind="ExternalInput")
    yk = nc.dram_tensor("yk", [128, 2048], bf16, kind="ExternalInput")
    yv = nc.dram_tensor("yv", [128, 2048], bf16, kind="ExternalInput")
    wq = nc.dram_tensor("wq", [128, 512], bf16, kind="ExternalInput")
    wkv = nc.dram_tensor("wkv", [128, 256], bf16, kind="ExternalInput")
    wp = nc.dram_tensor("wp", [128, 512], bf16, kind="ExternalInput")
    biases = nc.dram_tensor("biases", [1, 1152], bf16, kind="ExternalInput")
    bp = nc.dram_tensor("bp", [1, 512], f32, kind="ExternalInput")
    out = nc.dram_tensor("out", [2048, 256], bf16, kind="ExternalOutput")

    with tile.TileContext(nc) as tc:
        with (
            tc.tile_pool(name="const", bufs=1) as cp,
            tc.tile_pool(name="actt", bufs=3) as actt,
            tc.tile_pool(name="attp", bufs=16) as attp,
            tc.tile_pool(name="outp", bufs=3) as outp,
            tc.tile_pool(name="nrm", bufs=3) as nrm,
            tc.tile_pool(name="gp", bufs=3) as gp,
            tc.tile_pool(name="psc", bufs=4, space="PSUM") as psc,
            tc.tile_pool(name="pss", bufs=2, space="PSUM") as pss,
        ):
            # ---- load weights / inputs: one 2D DMA each, critical-first
            # per queue (sync/scalar/gpsimd each serialize their DMAs) ----
            def load1(t_dram, shape, tag, eng, dt=bf16, pieces=1):
                t = cp.tile(shape, dt, tag=tag, name=tag)
                w = shape[-1] // pieces
                for p in range(pieces):
                    eng.dma_start(
                        t[:, p * w : (p + 1) * w],
                        t_dram.ap()[:, p * w : (p + 1) * w])
                return t

            # big tensors split into ~128-256KB pieces: each DMA lands on
            # its own HW DMA engine (~55-60 GB/s each), so pieces transfer
            # in parallel even on one queue
            # q_conv(t2) needs xq cols [2t2*128,(2t2+2)*128) of BOTH c0
    
            # halves: first piece-pair rides ScalarE (before the ACT-table
            # preload), the rest ride sync behind wq/yk; biases tiny-first
            # on gpsimd so the trailing conv-bias matmuls never stall
            xqc = cp.tile([128, 4096], bf16, tag="xq", name="xq")
            xq_sb = [xqc[:, i * 2048 : (i + 1) * 2048] for i in range(2)]
            def xq_piece(p, eng):
                for i in range(2):
                    sl = slice(i * 2048 + p * 1024, i * 2048 + (p + 1) * 1024)
                    eng.dma_start(xqc[:, sl], xq.ap()[:, sl])
            xq_piece(0, nc.scalar)
            bias_sb = load1(biases, [1, 1152], "biases", nc.gpsimd)
            wkvc = load1(wkv, [128, 256], "wkv", nc.sync)
            ykc = load1(yk, [128, 2048], "yk", nc.sync, pieces=2)
            yvc = load1(yv, [128, 2048], "yv", nc.gpsimd, pieces=2)
            wpc = load1(wp, [128, 512], "wp", nc.gpsimd)
            bp_bc = cp.tile([128, 512], f32, tag="bp_bc", name="bp_bc")
            nc.gpsimd.dma_start(bp_bc[:], bp.ap().partition_broadcast(128))
            # xq is issued on ScalarE AFTER the ACT-table preload below, so
            # the yk transfers get the port first and the table DMA overlaps
            wkv_sb = [wkvc[:, i * 128 : (i + 1) * 128] for i in range(2)]
            yk_sb = [ykc[:, i * 1024 : (i + 1) * 1024] for i in range(2)]
            yv_sb = [yvc[:, i * 1024 : (i + 1) * 1024] for i in range(2)]
            wp_sb = [wpc[:, i * 256 : (i + 1) * 256] for i in range(2)]
            bq_sb = bias_sb[0:1, 0:512]
            bkv4_sb = bias_sb[0:1, 512:1024]
            bkvr_sb = bias_sb[0:1, 1024:1152]

            ones_row = cp.tile([1, 512], bf16, tag="ones", name="ones")
            nc.vector.memset(ones_row[:], 1.0)
            # preload the exp/tanh ACT table set during the input-DMA wait
            dum_t = actt.tile([128, 512], f32, tag="silu_t", name="dum_t")
            nc.scalar.activation(dum_t[0:1, :], ones_row[:], AF.Tanh, scale=0.5)
            wqc = load1(wq, [128, 512], "wq", nc.scalar)
            xq_piece(1, nc.sync)
            wq_sb = [wqc[:, i * 256 : (i + 1) * 256] for i in range(2)]



            # conv epilogue: psum z already includes bias (K=1 bias matmul).
            # t = tanh(z/2); u = z*t; dst = z + u = z*(1+tanh(z/2)) = 2silu(z)
            def silu_epi(ps, dst_ap, tag, rr=None):
                t = actt.tile([128, 512], f32, tag="silu_t", name=f"t_{tag}")
                u = actt.tile([128, 512], f32, tag="silu_u", name=f"u_{tag}")
                nc.scalar.activation(t[:], ps, AF.Tanh, scale=0.5)
                nc.vector.tensor_mul(u[:], ps, t[:])
                psv, uv = ps, u[:]
                if rr is not None:
                    psv = psv.rearrange(rr, a=4, h=2)
                    uv = uv.rearrange(rr, a=4, h=2)
                nc.vector.tensor_add(dst_ap, psv, uv)

            # ---- PE warmup on wkv (lands first): ~3.4us of full-array
            # matmuls inside the yk-DMA wait window flips the HAM clock
            # gate to 8/8 (2.4 GHz) before the real work starts.
            wps = psc.tile([128, 512], f32, tag="cnv", name="warm")
            for wi in range(16):
                nc.tensor.matmul(
                    wps[:, 0:256], lhsT=wkvc[:, 0:128], rhs=wkvc[:],
                    start=(wi == 0), stop=(wi == 15))

            # ---- kv conv (k part): kT[pp, m0, m1], pp = hl*64+d ----
            kT = cp.tile([128, 8, 128], adt, tag="kT")
            for mt in range(2):  # m0 quads
                ps = psc.tile([128, 512], f32, tag="cnv", name=f"psk{mt}")
                for mi in range(4):
                    m0 = 4 * mt + mi
                    for c0 in range(2):
                        nc.tensor.matmul(
                            ps[:, mi * 128 : (mi + 1) * 128],
                            lhsT=yk_sb[c0][:, m0 * 128 : (m0 + 1) * 128],
                            rhs=wkv_sb[c0],
                            start=(mi == 0 and c0 == 0), stop=False)
                nc.tensor.matmul(
                    ps[:], lhsT=ones_row[:, 0:128], rhs=bkv4_sb,
                    start=False, stop=True)
                silu_epi(
                    ps[:],
                    kT[:, 4 * mt : 4 * mt + 4, :].rearrange("p a b -> p (a b)"),
                    f"k{mt}")

            # ---- attention state (both heads) ----
            qT = cp.tile([128, 16, 256], adt, tag="qT")
            outun = [
                [cp.tile([128, 1024], adt, tag=f"outun{hl}_{i}",
                         name=f"outun{hl}_{i}") for i in range(2)]
                for hl in range(2)
            ]

            # q conv for one n0-pair
            def q_conv(t2):
                ps = psc.tile([128, 512], f32, tag="cnv", name=f"psq{t2}")
                for nn in range(2):
                    n0 = 2 * t2 + nn
                    for c0 in range(2):
                        nc.tensor.matmul(
                            ps[:, nn * 256 : (nn + 1) * 256],
                            lhsT=xq_sb[c0][:, n0 * 128 : (n0 + 1) * 128],
                            rhs=wq_sb[c0],
                            start=(nn == 0 and c0 == 0), stop=False)
                nc.tensor.matmul(
                    ps[:], lhsT=ones_row[:, 0:128], rhs=bq_sb,
                    start=False, stop=True)
                silu_epi(
                    ps[:],
                    qT[:, 2 * t2 : 2 * t2 + 2, :].rearrange("p a b -> p (a b)"),
                    f"q{t2}")

            q_conv(0)
            q_conv(1)

            # t2-outer attention. One [128,1024] score psum per m0 holds BOTH
            # heads (hl0 cols 0-511, hl1 cols 512-1023): the two score
            # matmuls target PE row-groups 0-63/64-127, become ready
            # together, and issue back-to-back -> they overlap in the array
            # (halving score time). Scores run one t2 AHEAD of exp so
            # ScalarE (the bottleneck) never waits.
            def scores_m0(t2, m0):
                scps = pss.tile([128, 1024], f32, tag="scp",
                                name=f"scp_{t2}_{m0}")
                for hl in range(2):
                    r0, r1 = hl * 64, (hl + 1) * 64
                    nc.tensor.matmul(
                        scps[:, hl * 512 : (hl + 1) * 512],
                        lhsT=kT[r0:r1, m0, :],
                        rhs=qT[r0:r1, 2 * t2 : 2 * t2 + 2, :],
                        start=True, stop=True)
                return scps

            def scores(t2):
                return [scores_m0(t2, m0) for m0 in range(8)]

            # ---- kv conv (v part): vext[m1, m0, hl, 0:64]=2v, [...,64]=2 ----
            vext = cp.tile([128, 8, 2, 65], adt, tag="vext")
            nc.vector.memset(vext[:], 2.0)
            for jv in range(2):
                ps = psc.tile([128, 512], f32, tag="cnv", name=f"psv{jv}")
                for c0 in range(2):
                    nc.tensor.matmul(
                        ps[:], lhsT=wkv_sb[c0],
                        rhs=yv_sb[c0][:, jv * 512 : (jv + 1) * 512],
                        start=(c0 == 0), stop=False)
                nc.tensor.matmul(
                    ps[:], lhsT=bkvr_sb, rhs=ones_row[:],
                    start=False, stop=True)
                silu_epi(
                    ps[:], vext[:, jv * 4 : (jv + 1) * 4, :, 0:64],
                    f"v{jv}", rr="p (a h d) -> p a h d")


            sc_cur = scores(0)
            for t2 in range(8):  # n0 pair (n0 = 2*t2 + nn)
                att = []
                for m0 in range(8):
                    a = attp.tile([128, 1024], adt, tag="att",
                                  name=f"att_{t2}_{m0}")
                    # scoresT = 4*q.k ; want exp(q.k/8) -> scale 1/32
                    nc.scalar.activation(
                        a[:], sc_cur[m0][:], AF.Exp, scale=0.03125)
                    att.append(a)

                # software-pipelined q conv (PE work fills exp-wait)
                if t2 + 2 < 8:
                    q_conv(t2 + 2)

                # av: both heads interleaved by m0 so the PE chases exp
                # pairwise (never more than one att tile behind); the first
                # next-t2 score pairs slot between the last av pairs so
                # exp(t2+1,0) is ready the moment exp(t2,7) retires
                opss = {}
                for hl in range(2):
                    opss[hl] = psc.tile([128, 512], f32, tag="cnv",
                                        name=f"ops{hl}_{t2}")

                def av_pair(m0):
                    for hl in range(2):
                        nc.tensor.matmul(
                            opss[hl][0:65, :], lhsT=vext[:, m0, hl, :],
                            rhs=att[m0][:, hl * 512 : (hl + 1) * 512],
                            start=(m0 == 0), stop=(m0 == 7))


                for m0 in range(6):
                    av_pair(m0)
                if t2 + 1 < 8:
                    nxt = [scores_m0(t2 + 1, 0)]
                    av_pair(6)
                    nxt.append(scores_m0(t2 + 1, 1))
                    av_pair(7)
                    nxt += [scores_m0(t2 + 1, m0) for m0 in range(2, 8)]
                    sc_cur = nxt
                else:
                    av_pair(6)
                    av_pair(7)

                # normalize: 1/denom off psum row 64 (SBUF hop), partition-
                # broadcast, multiply folded into the PSUM evacuation; at odd
                # t2 each head's projection follows its normalize directly so
                # proj(hl0) overlaps normalize(hl1)
                for hl in range(2):
                    ops = opss[hl]
                    # recip_approx_fast is wrong straight off PSUM (HW bit-
                    # trick breaks on the PSUM read path) — copy to SBUF
                    # first; at the last t2 ScalarE is idle, use it instead
                    drow = nrm.tile([1, 512], f32, tag="drow",
                                    name=f"drow{hl}_{t2}")
                    if t2 == 7:
                        nc.scalar.copy(drow[:], ops[64:65, :])
                    else:
                        nc.vector.tensor_copy(drow[:], ops[64:65, :])
                    rrow = nrm.tile([1, 512], f32, tag="rrow",
                                    name=f"rrow{hl}_{t2}")
                    nc.vector.reciprocal_approx_fast(
                        out=rrow[:], in_=drow[:])
                    g = gp.tile([64, 512], f32, tag="g", name=f"g{hl}_{t2}")
                    c0 = t2 % 2
                    q0 = t2 // 2
                    for nn in range(2):
                        # broadcast + evacuate per 256-col half so the first
                        # multiply starts while the second half broadcasts
                        sl = slice(nn * 256, (nn + 1) * 256)
                        nc.gpsimd.partition_broadcast(
                            g[0:64, sl], rrow[0:1, sl], channels=64)
                        # o * (1/denom) -> outun[c][band, q0-major cols]
                        dst = outun[hl][c0][
                            nn * 64 : nn * 64 + 64,
                            q0 * 256 : (q0 + 1) * 256]
                        nc.vector.tensor_mul(
                            dst,
                            ops[0:64, nn * 256 : (nn + 1) * 256],
                            g[0:64, sl])

                    if t2 % 2 == 0:
                        continue
                    # ---- quarter q0 complete for this head: projection ----
                    # proj fc in {2q0, 2q0+1}; rows hl*1024+half*512+q0+4r
                    ps2 = psc.tile([128, 512], f32, tag="cnv",
                                   name=f"psproj{hl}_{q0}")
                    for half in range(2):
                        fc = 2 * q0 + half
                        for cc in range(2):
                            nc.tensor.matmul(
                                ps2[:, half * 256 : (half + 1) * 256],
                                lhsT=outun[hl][cc][
                                    :, fc * 128 : (fc + 1) * 128],
                                rhs=wp_sb[cc],
                                start=(cc == 0), stop=(cc == 1))
                    osb = outp.tile([128, 512], bf16, tag="osb",
                                    name=f"osb{hl}_{q0}")
                    if q0 < 3:
                        nc.vector.tensor_add(osb[:], ps2[:], bp_bc[:])
                        dstap = bass.AP(
                            tensor=out,
                            offset=(hl * 1024 + q0) * 256,
                            ap=[[4 * 256, 128], [512 * 256, 2], [1, 256]])
                        nc.sync.dma_start(
                            dstap,
                            osb[:].rearrange("p (h c) -> p h c", h=2))
                    else:
                        # last quarter: per-half add+DMA on two queues so the
                        # final (tail-critical) output transfer is halved
                        for h in range(2):
                            sl = slice(h * 256, (h + 1) * 256)
                            nc.vector.tensor_add(
                                osb[:, sl], ps2[:, sl], bp_bc[:, sl])
                            dstap = bass.AP(
                                tensor=out,
                                offset=(hl * 1024 + q0 + h * 512) * 256,
                                ap=[[4 * 256, 128], [1, 256]])
                            eng = nc.sync if h == 0 else nc.scalar
                            eng.dma_start(dstap, osb[:, sl])

    nc.compile()
    return nc


def _prep_inputs(x, y, q_w, q_gamma, q_beta, q_mean, q_var,
                 kv_w, kv_gamma, kv_beta, kv_mean, kv_var, proj_w, proj_b):
    f = np.float32
    bf = ml_dtypes.bfloat16
    x = np.ascontiguousarray(np.asarray(x, f))
    y = np.ascontiguousarray(np.asarray(y, f))

    gq = np.asarray(q_gamma, f) / np.sqrt(np.asarray(q_var, f) + BN_EPS)
    bq_full = np.asarray(q_beta, f) - np.asarray(q_mean, f) * gq
    wq_host = np.ascontiguousarray((np.asarray(q_w, f) * gq[:, None]).T).astype(bf)

    gkv = np.asarray(kv_gamma, f) / np.sqrt(np.asarray(kv_var, f) + BN_EPS)
    bkv_full = np.asarray(kv_beta, f) - np.asarray(kv_mean, f) * gkv
    wkv_host = np.ascontiguousarray((np.asarray(kv_w, f) * gkv[:, None]).T).astype(bf)

    wp_host = np.ascontiguousarray(np.asarray(proj_w, f).T).astype(bf)
    bp_host = np.asarray(proj_b, f)

    bq2 = np.tile(bq_full[None, :], (1, 2)).astype(bf)
    bkv4_h = np.tile(bkv_full[None, :], (1, 4)).astype(bf)
    bkvr_h = bkv_full[None, :].astype(bf)
    bp2 = np.tile(bp_host[None, :], (1, 2)).astype(f)

    def fold(a):
        # [256, W] -> [128, 2W]: row halves side by side (one 2D DMA)
        return np.ascontiguousarray(
            np.concatenate([a[:128], a[128:]], axis=1))

    biases = np.concatenate([bq2, bkv4_h, bkvr_h], axis=1).astype(bf)
    wq_f = fold(wq_host)
    wkv_f = fold(wkv_host)
    wp_f = fold(wp_host)

    in_maps = []
    for core in range(8):
        b, hp = core // 2, core % 2
        X4 = x[b].reshape(C, N_TOK)
        Y4 = y[b].reshape(C, N_TOK)
        xqa = np.ascontiguousarray(
            X4.reshape(C, 16, 256)[:, :, hp * 128 : (hp + 1) * 128]
        ).reshape(C, 2048).astype(bf)
        Y8 = Y4.reshape(C, 8, 512)
        yka = np.ascontiguousarray(
            Y8[:, :, hp * 128 : (hp + 1) * 128]).reshape(C, 1024).astype(bf)
        yva = np.ascontiguousarray(
            Y8[:, :, 256 + hp * 128 : 256 + (hp + 1) * 128]
        ).reshape(C, 1024).astype(bf)
        in_maps.append({
            "xq": fold(xqa), "yk": fold(yka), "yv": fold(yva),
            "wq": wq_f, "wkv": wkv_f, "wp": wp_f,
            "biases": biases, "bp": bp2,
        })
    return in_maps


def _get_nc():
    if "nc" not in _CACHE:
        _CACHE["nc"] = _build()
    return _CACHE["nc"]


def kernel(x, y, H=64, W=64, q_w=None, q_gamma=None, q_beta=None, q_mean=None,
           q_var=None, kv_w=None, kv_gamma=None, kv_beta=None, kv_mean=None,
           kv_var=None, proj_w=None, proj_b=None, _trace=False):
    from concourse.bass_utils import run_bass_kernel_spmd

    nc = _get_nc()
    in_maps = _prep_inputs(x, y, q_w, q_gamma, q_beta, q_mean, q_var,
                           kv_w, kv_gamma, kv_beta, kv_mean, kv_var,
                           proj_w, proj_b)
    kw = {}
    if _trace:
        kw = {"trace": True, "trace_cores": list(range(8))}
    res = run_bass_kernel_spmd(nc, in_maps, list(range(8)), **kw)
    outa = np.empty((B, N_TOK, C), np.float32)
    for core in range(8):
        b, hp = core // 2, core % 2
        outa[b, hp * 2048 : (hp + 1) * 2048, :] = res.results[core]["out"]
    if _trace:
        return outa, res
    return outa


# revision 43
# speedup vs baseline: 1.0020x; 1.0020x over previous
"""Trainium2 Bass kernel for nn_Attention_57423712748130.

Computation (per batch b):
  X4 = x[b] viewed (C=256, N=4096)   [raw reshape]
  Q4 = silu(BN(q_w @ X4))            (256, 4096)
  KV4 = silu(BN(kv_w @ Y4))          (128, 4096)
  q[n,h,d]  = Q4[n1, n0*256+h*64+d]      n = n1*16+n0
  k[m,h,d]  = KV4[m1, m0*512 + h*64+d]   m = m1*8+m0
  v[m,h,d]  = KV4[m1, m0*512+256+h*64+d]
  att = softmax(q k^T / 8); o = att v
  out rows [h*1024,(h+1)*1024) = O_h @ proj_w.T + proj_b
    where O_h[n2, n3*64+d] = o[4*n2+n3, d]

Sharding: 8 cores = (batch b in 0..3) x (head-pair hp in 0..1); each core
computes heads {2hp, 2hp+1} of batch b = rows [hp*2048,(hp+1)*2048) of out[b].

On-core strategy (v2 — ScalarE(exp)-bound design, ~all else overlapped):
 - all bulk inputs/weights arrive as bf16 host-prefolded to [128, 2W]
   (row halves side by side) and load as 2D piece-DMAs spread over the 3
   DMA queues (sync/scalar/gpsimd; ~2.6us serial latency per DMA/queue,
   ~230 GB/s shared port); output returns bf16, upcast on host
 - conv outputs are computed directly in transposed layout so q^T/k^T
   need no transposes; conv bias is a trailing K=1 matmul (ones (x)
   bias_row) in the same PSUM accumulation; silu computed as
   z*(1+tanh(z/2)) = 2*silu(z) (tanh shares the ACT table set with exp,
   preloaded via a dummy op at t=0); the 2x is folded into the exp scale
   (1/32) and the 2.0 ones-column of V
 - a ~3.4us warmup burst of full-array matmuls on the first-landed
   weight flips the HAM clock gate to 8/8 (2.4 GHz) before real work
 - scoresT[m,n]: ONE [128,1024] psum per m0-block holds BOTH heads side
   by side, so the two score matmuls (PE row groups 0-63/64-127) become
   ready together and overlap in the array; one [128,1024] exp each;
   scores run one t2 AHEAD of exp so ScalarE (the bottleneck: 64 exp +
   12 tanh ops) never waits; av interleaves both heads by m0, chasing
   exp pairwise
 - att@v contracts over m with an extra 2.0-column on V producing
   softmax denominators as psum row 64; 1/denom = one DVE copy to SBUF
   (reciprocal_approx_fast reads garbage from PSUM on HW) + one
   reciprocal_approx_fast, gpsimd partition_broadcast per 256-col half,
   multiply folded into the PSUM-evacuation copy — no DRAM bounce
 - normalization + per-head projection run per q0-quarter, pipelined
   behind the attention of later quarters; O_h columns kept q0-major so
   every PSUM evacuation is contiguous; the output DMA un-permutes rows
"""

import ml_dtypes
import numpy as np

B = 4
N_TOK = 4096
C = 256
BN_EPS = 1e-5

_CACHE = {}


def _build():
    import concourse.bacc as bacc
    import concourse.bass as bass
    import concourse.tile as tile
    from concourse import mybir

    f32 = mybir.dt.float32
    bf16 = mybir.dt.bfloat16
    adt = bf16
    AF = mybir.ActivationFunctionType

    nc = bacc.Bacc("TRN2", target_bir_lowering=False, debug=False, num_devices=8)

    # all [256,W] operands arrive host-prefolded to [128, 2W] (row half
    # c0 side by side) so each loads with ONE 2D DMA; bf16 bias rows are
    # packed into one tensor (per-DMA queue latency is ~2.5-3us, serial)
    xq = nc.dram_tensor("xq", [128, 4096], bf16, kind="ExternalInput")
    yk = nc.dram_tensor("yk", [128, 2048], bf16, kind="ExternalInput")
    yv = nc.dram_tensor("yv", [128, 2048], bf16, kind="ExternalInput")
    wq = nc.dram_tensor("wq", [128, 512], bf16, kind="ExternalInput")
    wkv = nc.dram_tensor("wkv", [128, 256], bf16, kind="ExternalInput")
    wp = nc.dram_tensor("wp", [128, 512], bf16, kind="ExternalInput")
    biases = nc.dram_tensor("biases", [1, 1152], bf16, kind="ExternalInput")
    bp = nc.dram_tensor("bp", [1, 512], f32, kind="ExternalInput")
    out = nc.dram_tensor("out", [2048, 256], bf16, kind="ExternalOutput")

    with tile.TileContext(nc) as tc:
        with (
            tc.tile_pool(name="const", bufs=1) as cp,
            tc.tile_pool(name="actt", bufs=3) as actt,
            tc.tile_pool(name="attp", bufs=16) as attp,
            tc.tile_pool(name="outp", bufs=3) as outp,
            tc.tile_pool(name="nrm", bufs=3) as nrm,
            tc.tile_pool(name="gp", bufs=3) as gp,
            tc.tile_pool(name="psc", bufs=4, space="PSUM") as psc,
            tc.tile_pool(name="pss", bufs=2, space="PSUM") as pss,
        ):
            # ---- load weights / inputs: one 2D DMA each, critical-first
            # per queue (sync/scalar/gpsimd each serialize their DMAs) ----
            def load1(t_dram, shape, tag, eng, dt=bf16, pieces=1):
                t = cp.tile(shape, dt, tag=tag, name=tag)
                w = shape[-1] // pieces
                for p in range(pieces):
                    eng.dma_start(
                        t[:, p * w : (p + 1) * w],
                        t_dram.ap()[:, p * w : (p + 1) * w])
                return t

            # big tensors split into ~128-256KB pieces: each DMA lands on
            # its own HW DMA engine (~55-60 GB/s each), so pieces transfer
            # in parallel even on one queue
            # q_conv(t2) needs xq cols [2t2*128,(2t2+2)*128) of BOTH c0
    
            # halves: first piece-pair rides ScalarE (before the ACT-table
            # preload), the rest ride sync behind wq/yk; biases tiny-first
            # on gpsimd so the trailing conv-bias matmuls never stall
            xqc = cp.tile([128, 4096], bf16, tag="xq", name="xq")
            xq_sb = [xqc[:, i * 2048 : (i + 1) * 2048] for i in range(2)]
            def xq_piece(p, eng):
                for i in range(2):
                    sl = slice(i * 2048 + p * 1024, i * 2048 + (p + 1) * 1024)
                    eng.dma_start(xqc[:, sl], xq.ap()[:, sl])
            xq_piece(0, nc.scalar)
            bias_sb = load1(biases, [1, 1152], "biases", nc.gpsimd)
            wkvc = load1(wkv, [128, 256], "wkv", nc.sync)
            ykc = load1(yk, [128, 2048], "yk", nc.sync, pieces=2)
            yvc = load1(yv, [128, 2048], "yv", nc.gpsimd, pieces=2)
            wpc = load1(wp, [128, 512], "wp", nc.gpsimd)
            bp_bc = cp.tile([128, 512], f32, tag="bp_bc", name="bp_bc")
            nc.gpsimd.dma_start(bp_bc[:], bp.ap().partition_broadcast(128))
            # xq is issued on ScalarE AFTER the ACT-table preload below, so
            # the yk transfers get the port first and the table DMA overlaps
            wkv_sb = [wkvc[:, i * 128 : (i + 1) * 128] for i in range(2)]
            yk_sb = [ykc[:, i * 1024 : (i + 1) * 1024] for i in range(2)]
            yv_sb = [yvc[:, i * 1024 : (i + 1) * 1024] for i in range(2)]
            wp_sb = [wpc[:, i * 256 : (i + 1) * 256] for i in range(2)]
            bq_sb = bias_sb[0:1, 0:512]
            bkv4_sb = bias_sb[0:1, 512:1024]
            bkvr_sb = bias_sb[0:1, 1024:1152]

            ones_row = cp.tile([1, 512], bf16, tag="ones", name="ones")
            nc.vector.memset(ones_row[:], 1.0)
            # preload the exp/tanh ACT table set during the input-DMA wait
            dum_t = actt.tile([128, 512], f32, tag="silu_t", name="dum_t")
            nc.scalar.activation(dum_t[0:1, :], ones_row[:], AF.Tanh, scale=0.5)
            wqc = load1(wq, [128, 512], "wq", nc.scalar)
            xq_piece(1, nc.sync)
            wq_sb = [wqc[:, i * 256 : (i + 1) * 256] for i in range(2)]



            # conv epilogue: psum z already includes bias (K=1 bias matmul).
            # t = tanh(z/2); u = z*t; dst = z + u = z*(1+tanh(z/2)) = 2silu(z)
            def silu_epi(ps, dst_ap, tag, rr=None):
                t = actt.tile([128, 512], f32, tag="silu_t", name=f"t_{tag}")
                u = actt.tile([128, 512], f32, tag="silu_u", name=f"u_{tag}")
                nc.scalar.activation(t[:], ps, AF.Tanh, scale=0.5)
                nc.vector.tensor_mul(u[:], ps, t[:])
                psv, uv = ps, u[:]
                if rr is not None:
                    psv = psv.rearrange(rr, a=4, h=2)
                    uv = uv.rearrange(rr, a=4, h=2)
                nc.vector.tensor_add(dst_ap, psv, uv)

            # ---- PE warmup on wkv (lands first): ~3.4us of full-array
            # matmuls inside the yk-DMA wait window flips the HAM clock
            # gate to 8/8 (2.4 GHz) before the real work starts.
            wps = psc.tile([128, 512], f32, tag="cnv", name="warm")
            for wi in range(16):
                nc.tensor.matmul(
                    wps[:, 0:256], lhsT=wkvc[:, 0:128], rhs=wkvc[:],
                    start=(wi == 0), stop=(wi == 15))

            # ---- kv conv (k part): kT[pp, m0, m1], pp = hl*64+d ----
            kT = cp.tile([128, 8, 128], adt, tag="kT")
            for mt in range(2):  # m0 quads
                ps = psc.tile([128, 512], f32, tag="cnv", name=f"psk{mt}")
                for mi in range(4):
                    m0 = 4 * mt + mi
                    for c0 in range(2):
                        nc.tensor.matmul(
                            ps[:, mi * 128 : (mi + 1) * 128],
                            lhsT=yk_sb[c0][:, m0 * 128 : (m0 + 1) * 128],
                            rhs=wkv_sb[c0],
                            start=(mi == 0 and c0 == 0), stop=False)
                nc.tensor.matmul(
                    ps[:], lhsT=ones_row[:, 0:128], rhs=bkv4_sb,
                    start=False, stop=True)
                silu_epi(
                    ps[:],
                    kT[:, 4 * mt : 4 * mt + 4, :].rearrange("p a b -> p (a b)"),
                    f"k{mt}")

            # ---- attention state (both heads) ----
            qT = cp.tile([128, 16, 256], adt, tag="qT")
            outun = [
                [cp.tile([128, 1024], adt, tag=f"outun{hl}_{i}",
                         name=f"outun{hl}_{i}") for i in range(2)]
                for hl in range(2)
            ]

            # q conv for one n0-pair
            def q_conv(t2):
                ps = psc.tile([128, 512], f32, tag="cnv", name=f"psq{t2}")
                for nn in range(2):
                    n0 = 2 * t2 + nn
                    for c0 in range(2):
                        nc.tensor.matmul(
                            ps[:, nn * 256 : (nn + 1) * 256],
                            lhsT=xq_sb[c0][:, n0 * 128 : (n0 + 1) * 128],
                            rhs=wq_sb[c0],
                            start=(nn == 0 and c0 == 0), stop=False)
                nc.tensor.matmul(
                    ps[:], lhsT=ones_row[:, 0:128], rhs=bq_sb,
                    start=False, stop=True)
                silu_epi(
                    ps[:],
                    qT[:, 2 * t2 : 2 * t2 + 2, :].rearrange("p a b -> p (a b)"),
                    f"q{t2}")

            q_conv(0)
            q_conv(1)

            # t2-outer attention. One [128,1024] score psum per m0 holds BOTH
            # heads (hl0 cols 0-511, hl1 cols 512-1023): the two score
            # matmuls target PE row-groups 0-63/64-127, become ready
            # together, and issue back-to-back -> they overlap in the array
            # (halving score time). Scores run one t2 AHEAD of exp so
            # ScalarE (the bottleneck) never waits.
            def scores_m0(t2, m0):
                scps = pss.tile([128, 1024], f32, tag="scp",
                                name=f"scp_{t2}_{m0}")
                for hl in range(2):
                    r0, r1 = hl * 64, (hl + 1) * 64
                    nc.tensor.matmul(
                        scps[:, hl * 512 : (hl + 1) * 512],
                        lhsT=kT[r0:r1, m0, :],
                        rhs=qT[r0:r1, 2 * t2 : 2 * t2 + 2, :],
                        start=True, stop=True)
                return scps

            def scores(t2):
                return [scores_m0(t2, m0) for m0 in range(8)]

            # ---- kv conv (v part): vext[m1, m0, hl, 0:64]=2v, [...,64]=2 ----
            vext = cp.tile([128, 8, 2, 65], adt, tag="vext")
            nc.vector.memset(vext[:], 2.0)
            for jv in range(2):
                ps = psc.tile([128, 512], f32, tag="cnv", name=f"psv{jv}")
                for c0 in range(2):
                    nc.tensor.matmul(
                        ps[:], lhsT=wkv_sb[c0],
                        rhs=yv_sb[c0][:, jv * 512 : (jv + 1) * 512],
                        start=(c0 == 0), stop=False)
                nc.tensor.matmul(
                    ps[:], lhsT=bkvr_sb, rhs=ones_row[:],
                    start=False, stop=True)
                silu_epi(
                    ps[:], vext[:, jv * 4 : (jv + 1) * 4, :, 0:64],
                    f"v{jv}", rr="p (a h d) -> p a h d")


            sc_cur = scores(0)
            for t2 in range(8):  # n0 pair (n0 = 2*t2 + nn)
                att = []
                for m0 in range(8):
                    a = attp.tile([128, 1024], adt, tag="att",
                                  name=f"att_{t2}_{m0}")
                    # scoresT = 4*q.k ; want exp(q.k/8) -> scale 1/32
                    nc.scalar.activation(
                        a[:], sc_cur[m0][:], AF.Exp, scale=0.03125)
                    att.append(a)

                # software-pipelined q conv (PE work fills exp-wait)
                if t2 + 2 < 8:
                    q_conv(t2 + 2)

                # av: both heads interleaved by m0 so the PE chases exp
                # pairwise (never more than one att tile behind); the first
                # next-t2 score pairs slot between the last av pairs so
                # exp(t2+1,0) is ready the moment exp(t2,7) retires
                opss = {}
                for hl in range(2):
                    opss[hl] = psc.tile([128, 512], f32, tag="cnv",
                                        name=f"ops{hl}_{t2}")

                def av_pair(m0):
                    for hl in range(2):
                        nc.tensor.matmul(
                            opss[hl][0:65, :], lhsT=vext[:, m0, hl, :],
                            rhs=att[m0][:, hl * 512 : (hl + 1) * 512],
                            start=(m0 == 0), stop=(m0 == 7))
                    # HAM keep-warm: standalone weight loads (PE-array
                    # activity, no PSUM) gated on the same att tile, padding
                    # PE duty through the exp window so the clock gate
                    # stays at 8/8; orphaned loads are overwritten by the
                    # next matmul's own LDWEIGHTS
                    for w0 in range(2):
                        nc.tensor.ldweights(
                            att[m0][:, w0 * 128 : (w0 + 1) * 128])


                for m0 in range(6):
                    av_pair(m0)
                if t2 + 1 < 8:
                    nxt = [scores_m0(t2 + 1, 0)]
                    av_pair(6)
                    nxt.append(scores_m0(t2 + 1, 1))
                    av_pair(7)
                    nxt += [scores_m0(t2 + 1, m0) for m0 in range(2, 8)]
                    sc_cur = nxt
                else:
                    av_pair(6)
                    av_pair(7)

                # normalize: 1/denom off psum row 64 (SBUF hop), partition-
                # broadcast, multiply folded into the PSUM evacuation; at odd
                # t2 each head's projection follows its normalize directly so
                # proj(hl0) overlaps normalize(hl1)
                for hl in range(2):
                    ops = opss[hl]
                    # recip_approx_fast is wrong straight off PSUM (HW bit-
                    # trick breaks on the PSUM read path) — copy to SBUF
                    # first; at the last t2 ScalarE is idle, use it instead
                    drow = nrm.tile([1, 512], f32, tag="drow",
                                    name=f"drow{hl}_{t2}")
                    if t2 == 7:
                        nc.scalar.copy(drow[:], ops[64:65, :])
                    else:
                        nc.vector.tensor_copy(drow[:], ops[64:65, :])
                    rrow = nrm.tile([1, 512], f32, tag="rrow",
                                    name=f"rrow{hl}_{t2}")
                    nc.vector.reciprocal_approx_fast(
                        out=rrow[:], in_=drow[:])
                    g = gp.tile([64, 512], f32, tag="g", name=f"g{hl}_{t2}")
                    c0 = t2 % 2
                    q0 = t2 // 2
                    for nn in range(2):
                        # broadcast + evacuate per 256-col half so the first
                        # multiply starts while the second half broadcasts
                        sl = slice(nn * 256, (nn + 1) * 256)
                        nc.gpsimd.partition_broadcast(
                            g[0:64, sl], rrow[0:1, sl], channels=64)
                        # o * (1/denom) -> outun[c][band, q0-major cols]
                        dst = outun[hl][c0][
                            nn * 64 : nn * 64 + 64,
                            q0 * 256 : (q0 + 1) * 256]
                        nc.vector.tensor_mul(
                            dst,
                            ops[0:64, nn * 256 : (nn + 1) * 256],
                            g[0:64, sl])

                    if t2 % 2 == 0:
                        continue
                    # ---- quarter q0 complete for this head: projection ----
                    # proj fc in {2q0, 2q0+1}; rows hl*1024+half*512+q0+4r
                    ps2 = psc.tile([128, 512], f32, tag="cnv",
                                   name=f"psproj{hl}_{q0}")
                    for half in range(2):
                        fc = 2 * q0 + half
                        for cc in range(2):
                            nc.tensor.matmul(
                                ps2[:, half * 256 : (half + 1) * 256],
                                lhsT=outun[hl][cc][
                                    :, fc * 128 : (fc + 1) * 128],
                                rhs=wp_sb[cc],
                                start=(cc == 0), stop=(cc == 1))
                    osb = outp.tile([128, 512], bf16, tag="osb",
                                    name=f"osb{hl}_{q0}")
                    if q0 < 3:
                        nc.vector.tensor_add(osb[:], ps2[:], bp_bc[:])
                        dstap = bass.AP(
                            tensor=out,
                            offset=(hl * 1024 + q0) * 256,
                            ap=[[4 * 256, 128], [512 * 256, 2], [1, 256]])
                        nc.sync.dma_start(
                            dstap,
                            osb[:].rearrange("p (h c) -> p h c", h=2))
                    else:
                        # last quarter: per-half add+DMA on two queues so the
                        # final (tail-critical) output transfer is halved
                        for h in range(2):
                            sl = slice(h * 256, (h + 1) * 256)
                            nc.vector.tensor_add(
                                osb[:, sl], ps2[:, sl], bp_bc[:, sl])
                            dstap = bass.AP(
                                tensor=out,
                                offset=(hl * 1024 + q0 + h * 512) * 256,
                                ap=[[4 * 256, 128], [1, 256]])
                            eng = nc.sync if h == 0 else nc.scalar
                            eng.dma_start(dstap, osb[:, sl])

    nc.compile()
    return nc


def _prep_inputs(x, y, q_w, q_gamma, q_beta, q_mean, q_var,
                 kv_w, kv_gamma, kv_beta, kv_mean, kv_var, proj_w, proj_b):
    f = np.float32
    bf = ml_dtypes.bfloat16
    x = np.ascontiguousarray(np.asarray(x, f))
    y = np.ascontiguousarray(np.asarray(y, f))

    gq = np.asarray(q_gamma, f) / np.sqrt(np.asarray(q_var, f) + BN_EPS)
    bq_full = np.asarray(q_beta, f) - np.asarray(q_mean, f) * gq
    wq_host = np.ascontiguousarray((np.asarray(q_w, f) * gq[:, None]).T).astype(bf)

    gkv = np.asarray(kv_gamma, f) / np.sqrt(np.asarray(kv_var, f) + BN_EPS)
    bkv_full = np.asarray(kv_beta, f) - np.asarray(kv_mean, f) * gkv
    wkv_host = np.ascontiguousarray((np.asarray(kv_w, f) * gkv[:, None]).T).astype(bf)

    wp_host = np.ascontiguousarray(np.asarray(proj_w, f).T).astype(bf)
    bp_host = np.asarray(proj_b, f)

    bq2 = np.tile(bq_full[None, :], (1, 2)).astype(bf)
    bkv4_h = np.tile(bkv_full[None, :], (1, 4)).astype(bf)
    bkvr_h = bkv_full[None, :].astype(bf)
    bp2 = np.tile(bp_host[None, :], (1, 2)).astype(f)

    def fold(a):
        # [256, W] -> [128, 2W]: row halves side by side (one 2D DMA)
        return np.ascontiguousarray(
            np.concatenate([a[:128], a[128:]], axis=1))

    biases = np.concatenate([bq2, bkv4_h, bkvr_h], axis=1).astype(bf)
    wq_f = fold(wq_host)
    wkv_f = fold(wkv_host)
    wp_f = fold(wp_host)

    in_maps = []
    for core in range(8):
        b, hp = core // 2, core % 2
        X4 = x[b].reshape(C, N_TOK)
        Y4 = y[b].reshape(C, N_TOK)
        xqa = np.ascontiguousarray(
            X4.reshape(C, 16, 256)[:, :, hp * 128 : (hp + 1) * 128]
        ).reshape(C, 2048).astype(bf)
        Y8 = Y4.reshape(C, 8, 512)
        yka = np.ascontiguousarray(
            Y8[:, :, hp * 128 : (hp + 1) * 128]).reshape(C, 1024).astype(bf)
        yva = np.ascontiguousarray(
            Y8[:, :, 256 + hp * 128 : 256 + (hp + 1) * 128]
        ).reshape(C, 1024).astype(bf)
        in_maps.append({
            "xq": fold(xqa), "yk": fold(yka), "yv": fold(yva),
            "wq": wq_f, "wkv": wkv_f, "wp": wp_f,
            "biases": biases, "bp": bp2,
        })
    return in_maps


def _get_nc():
    if "nc" not in _CACHE:
        _CACHE["nc"] = _build()
    return _CACHE["nc"]


def kernel(x, y, H=64, W=64, q_w=None, q_gamma=None, q_beta=None, q_mean=None,
           q_var=None, kv_w=None, kv_gamma=None, kv_beta=None, kv_mean=None,
           kv_var=None, proj_w=None, proj_b=None, _trace=False):
    from concourse.bass_utils import run_bass_kernel_spmd

    nc = _get_nc()
    in_maps = _prep_inputs(x, y, q_w, q_gamma, q_beta, q_mean, q_var,
                           kv_w, kv_gamma, kv_beta, kv_mean, kv_var,
                           proj_w, proj_b)
    kw = {}
    if _trace:
        kw = {"trace": True, "trace_cores": list(range(8))}
    res = run_bass_kernel_spmd(nc, in_maps, list(range(8)), **kw)
    outa = np.empty((B, N_TOK, C), np.float32)
    for core in range(8):
        b, hp = core // 2, core % 2
        outa[b, hp * 2048 : (hp + 1) * 2048, :] = res.results[core]["out"]
    if _trace:
        return outa, res
    return outa


# revision 44
# speedup vs baseline: 1.0174x; 1.0154x over previous
"""Trainium2 Bass kernel for nn_Attention_57423712748130.

Computation (per batch b):
  X4 = x[b] viewed (C=256, N=4096)   [raw reshape]
  Q4 = silu(BN(q_w @ X4))            (256, 4096)
  KV4 = silu(BN(kv_w @ Y4))          (128, 4096)
  q[n,h,d]  = Q4[n1, n0*256+h*64+d]      n = n1*16+n0
  k[m,h,d]  = KV4[m1, m0*512 + h*64+d]   m = m1*8+m0
  v[m,h,d]  = KV4[m1, m0*512+256+h*64+d]
  att = softmax(q k^T / 8); o = att v
  out rows [h*1024,(h+1)*1024) = O_h @ proj_w.T + proj_b
    where O_h[n2, n3*64+d] = o[4*n2+n3, d]

Sharding: 8 cores = (batch b in 0..3) x (head-pair hp in 0..1); each core
computes heads {2hp, 2hp+1} of batch b = rows [hp*2048,(hp+1)*2048) of out[b].

On-core strategy (v2 — ScalarE(exp)-bound design, ~all else overlapped):
 - all bulk inputs/weights arrive as bf16 host-prefolded to [128, 2W]
   (row halves side by side) and load as 2D piece-DMAs spread over the 3
   DMA queues (sync/scalar/gpsimd; ~2.6us serial latency per DMA/queue,
   ~230 GB/s shared port); output returns bf16, upcast on host
 - conv outputs are computed directly in transposed layout so q^T/k^T
   need no transposes; conv bias is a trailing K=1 matmul (ones (x)
   bias_row) in the same PSUM accumulation; silu computed as
   z*(1+tanh(z/2)) = 2*silu(z) (tanh shares the ACT table set with exp,
   preloaded via a dummy op at t=0); the 2x is folded into the exp scale
   (1/32) and the 2.0 ones-column of V
 - a ~3.4us warmup burst of full-array matmuls on the first-landed
   weight flips the HAM clock gate to 8/8 (2.4 GHz) before real work
 - scoresT[m,n]: ONE [128,1024] psum per m0-block holds BOTH heads side
   by side, so the two score matmuls (PE row groups 0-63/64-127) become
   ready together and overlap in the array; one [128,1024] exp each;
   scores run one t2 AHEAD of exp so ScalarE (the bottleneck: 64 exp +
   12 tanh ops) never waits; av interleaves both heads by m0, chasing
   exp pairwise
 - att@v contracts over m with an extra 2.0-column on V producing
   softmax denominators as psum row 64; 1/denom = one DVE copy to SBUF
   (reciprocal_approx_fast reads garbage from PSUM on HW) + one
   reciprocal_approx_fast, gpsimd partition_broadcast per 256-col half,
   multiply folded into the PSUM-evacuation copy — no DRAM bounce
 - normalization + per-head projection run per q0-quarter, pipelined
   behind the attention of later quarters; O_h columns kept q0-major so
   every PSUM evacuation is contiguous; the output DMA un-permutes rows
"""

import ml_dtypes
import numpy as np

B = 4
N_TOK = 4096
C = 256
BN_EPS = 1e-5

_CACHE = {}


def _build():
    import concourse.bacc as bacc
    import concourse.bass as bass
    import concourse.tile as tile
    from concourse import mybir

    f32 = mybir.dt.float32
    bf16 = mybir.dt.bfloat16
    adt = bf16
    AF = mybir.ActivationFunctionType

    nc = bacc.Bacc("TRN2", target_bir_lowering=False, debug=False, num_devices=8)

    # all [256,W] operands arrive host-prefolded to [128, 2W] (row half
    # c0 side by side) so each loads with ONE 2D DMA; bf16 bias rows are
    # packed into one tensor (per-DMA queue latency is ~2.5-3us, serial)
    xq = nc.dram_tensor("xq", [128, 4096], bf16, kind="ExternalInput")
    yk = nc.dram_tensor("yk", [128, 2048], bf16, kind="ExternalInput")
    yv = nc.dram_tensor("yv", [128, 2048], bf16, kind="ExternalInput")
    wq = nc.dram_tensor("wq", [128, 512], bf16, kind="ExternalInput")
    wkv = nc.dram_tensor("wkv", [128, 256], bf16, kind="ExternalInput")
    wp = nc.dram_tensor("wp", [128, 512], bf16, kind="ExternalInput")
    biases = nc.dram_tensor("biases", [1, 1152], bf16, kind="ExternalInput")
    bp = nc.dram_tensor("bp", [1, 512], f32, kind="ExternalInput")
    out = nc.dram_tensor("out", [2048, 256], bf16, kind="ExternalOutput")

    with tile.TileContext(nc) as tc:
        with (
            tc.tile_pool(name="const", bufs=1) as cp,
            tc.tile_pool(name="actt", bufs=3) as actt,
            tc.tile_pool(name="attp", bufs=24) as attp,
            tc.tile_pool(name="outp", bufs=3) as outp,
            tc.tile_pool(name="nrm", bufs=3) as nrm,
            tc.tile_pool(name="gp", bufs=3) as gp,
            tc.tile_pool(name="psc", bufs=4, space="PSUM") as psc,
            tc.tile_pool(name="pss", bufs=2, space="PSUM") as pss,
        ):
            # ---- load weights / inputs: one 2D DMA each, critical-first
            # per queue (sync/scalar/gpsimd each serialize their DMAs) ----
            def load1(t_dram, shape, tag, eng, dt=bf16, pieces=1):
                t = cp.tile(shape, dt, tag=tag, name=tag)
                w = shape[-1] // pieces
                for p in range(pieces):
                    eng.dma_start(
                        t[:, p * w : (p + 1) * w],
                        t_dram.ap()[:, p * w : (p + 1) * w])
                return t

            # big tensors split into ~128-256KB pieces: each DMA lands on
            # its own HW DMA engine (~55-60 GB/s each), so pieces transfer
            # in parallel even on one queue
            # q_conv(t2) needs xq cols [2t2*128,(2t2+2)*128) of BOTH c0
    
            # halves: first piece-pair rides ScalarE (before the ACT-table
            # preload), the rest ride sync behind wq/yk; biases tiny-first
            # on gpsimd so the trailing conv-bias matmuls never stall
            xqc = cp.tile([128, 4096], bf16, tag="xq", name="xq")
            xq_sb = [xqc[:, i * 2048 : (i + 1) * 2048] for i in range(2)]
            def xq_piece(p, eng):
                for i in range(2):
                    sl = slice(i * 2048 + p * 1024, i * 2048 + (p + 1) * 1024)
                    eng.dma_start(xqc[:, sl], xq.ap()[:, sl])
            xq_piece(0, nc.scalar)
            bias_sb = load1(biases, [1, 1152], "biases", nc.gpsimd)
            wkvc = load1(wkv, [128, 256], "wkv", nc.sync)
            ykc = load1(yk, [128, 2048], "yk", nc.sync, pieces=2)
            yvc = load1(yv, [128, 2048], "yv", nc.gpsimd, pieces=2)
            wpc = load1(wp, [128, 512], "wp", nc.gpsimd)
            bp_bc = cp.tile([128, 512], f32, tag="bp_bc", name="bp_bc")
            nc.gpsimd.dma_start(bp_bc[:], bp.ap().partition_broadcast(128))
            # xq is issued on ScalarE AFTER the ACT-table preload below, so
            # the yk transfers get the port first and the table DMA overlaps
            wkv_sb = [wkvc[:, i * 128 : (i + 1) * 128] for i in range(2)]
            yk_sb = [ykc[:, i * 1024 : (i + 1) * 1024] for i in range(2)]
            yv_sb = [yvc[:, i * 1024 : (i + 1) * 1024] for i in range(2)]
            wp_sb = [wpc[:, i * 256 : (i + 1) * 256] for i in range(2)]
            bq_sb = bias_sb[0:1, 0:512]
            bkv4_sb = bias_sb[0:1, 512:1024]
            bkvr_sb = bias_sb[0:1, 1024:1152]

            ones_row = cp.tile([1, 512], bf16, tag="ones", name="ones")
            nc.vector.memset(ones_row[:], 1.0)
            # preload the exp/tanh ACT table set during the input-DMA wait
            dum_t = actt.tile([128, 512], f32, tag="silu_t", name="dum_t")
            nc.scalar.activation(dum_t[0:1, :], ones_row[:], AF.Tanh, scale=0.5)
            wqc = load1(wq, [128, 512], "wq", nc.scalar)
            xq_piece(1, nc.sync)
            wq_sb = [wqc[:, i * 256 : (i + 1) * 256] for i in range(2)]



            # conv epilogue: psum z already includes bias (K=1 bias matmul).
            # t = tanh(z/2); u = z*t; dst = z + u = z*(1+tanh(z/2)) = 2silu(z)
            def silu_epi(ps, dst_ap, tag, rr=None):
                t = actt.tile([128, 512], f32, tag="silu_t", name=f"t_{tag}")
                u = actt.tile([128, 512], f32, tag="silu_u", name=f"u_{tag}")
                nc.scalar.activation(t[:], ps, AF.Tanh, scale=0.5)
                nc.vector.tensor_mul(u[:], ps, t[:])
                psv, uv = ps, u[:]
                if rr is not None:
                    psv = psv.rearrange(rr, a=4, h=2)
                    uv = uv.rearrange(rr, a=4, h=2)
                nc.vector.tensor_add(dst_ap, psv, uv)

            # ---- PE warmup on wkv (lands first): ~3.4us of full-array
            # matmuls inside the yk-DMA wait window flips the HAM clock
            # gate to 8/8 (2.4 GHz) before the real work starts.
            wps = psc.tile([128, 512], f32, tag="cnv", name="warm")
            for wi in range(9):
                nc.tensor.matmul(
                    wps[:], lhsT=xqc[:, 0:128], rhs=xqc[:, 0:512],
                    start=(wi == 0), stop=(wi == 8))

            # ---- kv conv (k part): kT[pp, m0, m1], pp = hl*64+d ----
            kT = cp.tile([128, 8, 128], adt, tag="kT")
            for mt in range(2):  # m0 quads
                ps = psc.tile([128, 512], f32, tag="cnv", name=f"psk{mt}")
                for mi in range(4):
                    m0 = 4 * mt + mi
                    for c0 in range(2):
                        nc.tensor.matmul(
                            ps[:, mi * 128 : (mi + 1) * 128],
                            lhsT=yk_sb[c0][:, m0 * 128 : (m0 + 1) * 128],
                            rhs=wkv_sb[c0],
                            start=(mi == 0 and c0 == 0), stop=False)
                nc.tensor.matmul(
                    ps[:], lhsT=ones_row[:, 0:128], rhs=bkv4_sb,
                    start=False, stop=True)
                silu_epi(
                    ps[:],
                    kT[:, 4 * mt : 4 * mt + 4, :].rearrange("p a b -> p (a b)"),
                    f"k{mt}")

            # ---- attention state (both heads) ----
            qT = cp.tile([128, 16, 256], adt, tag="qT")
            outun = [
                [cp.tile([128, 1024], adt, tag=f"outun{hl}_{i}",
                         name=f"outun{hl}_{i}") for i in range(2)]
                for hl in range(2)
            ]

            # q conv for one n0-pair
            def q_conv(t2):
                ps = psc.tile([128, 512], f32, tag="cnv", name=f"psq{t2}")
                for nn in range(2):
                    n0 = 2 * t2 + nn
                    for c0 in range(2):
                        nc.tensor.matmul(
                            ps[:, nn * 256 : (nn + 1) * 256],
                            lhsT=xq_sb[c0][:, n0 * 128 : (n0 + 1) * 128],
                            rhs=wq_sb[c0],
                            start=(nn == 0 and c0 == 0), stop=False)
                nc.tensor.matmul(
                    ps[:], lhsT=ones_row[:, 0:128], rhs=bq_sb,
                    start=False, stop=True)
                silu_epi(
                    ps[:],
                    qT[:, 2 * t2 : 2 * t2 + 2, :].rearrange("p a b -> p (a b)"),
                    f"q{t2}")

            q_conv(0)
            q_conv(1)

            # t2-outer attention. One [128,1024] score psum per m0 holds BOTH
            # heads (hl0 cols 0-511, hl1 cols 512-1023): the two score
            # matmuls target PE row-groups 0-63/64-127, become ready
            # together, and issue back-to-back -> they overlap in the array
            # (halving score time). Scores run one t2 AHEAD of exp so
            # ScalarE (the bottleneck) never waits.
            def scores_m0(t2, m0):
                scps = pss.tile([128, 1024], f32, tag="scp",
                                name=f"scp_{t2}_{m0}")
                for hl in range(2):
                    r0, r1 = hl * 64, (hl + 1) * 64
                    nc.tensor.matmul(
                        scps[:, hl * 512 : (hl + 1) * 512],
                        lhsT=kT[r0:r1, m0, :],
                        rhs=qT[r0:r1, 2 * t2 : 2 * t2 + 2, :],
                        start=True, stop=True)
                return scps

            def scores(t2):
                return [scores_m0(t2, m0) for m0 in range(8)]

            # ---- kv conv (v part): vext[m1, m0, hl, 0:64]=2v, [...,64]=2 ----
            vext = cp.tile([128, 8, 2, 65], adt, tag="vext")
            nc.vector.memset(vext[:], 2.0)
            for jv in range(2):
                ps = psc.tile([128, 512], f32, tag="cnv", name=f"psv{jv}")
                for c0 in range(2):
                    nc.tensor.matmul(
                        ps[:], lhsT=wkv_sb[c0],
                        rhs=yv_sb[c0][:, jv * 512 : (jv + 1) * 512],
                        start=(c0 == 0), stop=False)
                nc.tensor.matmul(
                    ps[:], lhsT=bkvr_sb, rhs=ones_row[:],
                    start=False, stop=True)
                silu_epi(
                    ps[:], vext[:, jv * 4 : (jv + 1) * 4, :, 0:64],
                    f"v{jv}", rr="p (a h d) -> p a h d")


            sc_cur = scores(0)
            for t2 in range(8):  # n0 pair (n0 = 2*t2 + nn)
                att = []
                for m0 in range(8):
                    a = attp.tile([128, 1024], adt, tag="att",
                                  name=f"att_{t2}_{m0}")
                    # scoresT = 4*q.k ; want exp(q.k/8) -> scale 1/32
                    nc.scalar.activation(
                        a[:], sc_cur[m0][:], AF.Exp, scale=0.03125)
                    att.append(a)

                # software-pipelined q conv (PE work fills exp-wait)
                if t2 + 2 < 8:
                    q_conv(t2 + 2)

                # av: both heads interleaved by m0 so the PE chases exp
                # pairwise (never more than one att tile behind); the first
                # next-t2 score pairs slot between the last av pairs so
                # exp(t2+1,0) is ready the moment exp(t2,7) retires
                opss = {}
                for hl in range(2):
                    opss[hl] = psc.tile([128, 512], f32, tag="cnv",
                                        name=f"ops{hl}_{t2}")

                def av_pair(m0):
                    for hl in range(2):
                        nc.tensor.matmul(
                            opss[hl][0:65, :], lhsT=vext[:, m0, hl, :],
                            rhs=att[m0][:, hl * 512 : (hl + 1) * 512],
                            start=(m0 == 0), stop=(m0 == 7))


                for m0 in range(6):
                    av_pair(m0)
                if t2 + 1 < 8:
                    nxt = [scores_m0(t2 + 1, 0)]
                    av_pair(6)
                    nxt.append(scores_m0(t2 + 1, 1))
                    av_pair(7)
                    nxt += [scores_m0(t2 + 1, m0) for m0 in range(2, 8)]
                    sc_cur = nxt
                else:
                    av_pair(6)
                    av_pair(7)

                # normalize: 1/denom off psum row 64 (SBUF hop), partition-
                # broadcast, multiply folded into the PSUM evacuation; at odd
                # t2 each head's projection follows its normalize directly so
                # proj(hl0) overlaps normalize(hl1)
                for hl in range(2):
                    ops = opss[hl]
                    # recip_approx_fast is wrong straight off PSUM (HW bit-
                    # trick breaks on the PSUM read path) — copy to SBUF
                    # first; at the last t2 ScalarE is idle, use it instead
                    drow = nrm.tile([1, 512], f32, tag="drow",
                                    name=f"drow{hl}_{t2}")
                    if t2 == 7:
                        nc.scalar.copy(drow[:], ops[64:65, :])
                    else:
                        nc.vector.tensor_copy(drow[:], ops[64:65, :])
                    rrow = nrm.tile([1, 512], f32, tag="rrow",
                                    name=f"rrow{hl}_{t2}")
                    nc.vector.reciprocal_approx_fast(
                        out=rrow[:], in_=drow[:])
                    g = gp.tile([64, 512], f32, tag="g", name=f"g{hl}_{t2}")
                    c0 = t2 % 2
                    q0 = t2 // 2
                    for nn in range(2):
                        # broadcast + evacuate per 256-col half so the first
                        # multiply starts while the second half broadcasts
                        sl = slice(nn * 256, (nn + 1) * 256)
                        nc.gpsimd.partition_broadcast(
                            g[0:64, sl], rrow[0:1, sl], channels=64)
                        # o * (1/denom) -> outun[c][band, q0-major cols]
                        dst = outun[hl][c0][
                            nn * 64 : nn * 64 + 64,
                            q0 * 256 : (q0 + 1) * 256]
                        nc.vector.tensor_mul(
                            dst,
                            ops[0:64, nn * 256 : (nn + 1) * 256],
                            g[0:64, sl])

                    if t2 % 2 == 0:
                        continue
                    # ---- quarter q0 complete for this head: projection ----
                    # proj fc in {2q0, 2q0+1}; rows hl*1024+half*512+q0+4r
                    ps2 = psc.tile([128, 512], f32, tag="cnv",
                                   name=f"psproj{hl}_{q0}")
                    for half in range(2):
                        fc = 2 * q0 + half
                        for cc in range(2):
                            nc.tensor.matmul(
                                ps2[:, half * 256 : (half + 1) * 256],
                                lhsT=outun[hl][cc][
                                    :, fc * 128 : (fc + 1) * 128],
                                rhs=wp_sb[cc],
                                start=(cc == 0), stop=(cc == 1))
                    osb = outp.tile([128, 512], bf16, tag="osb",
                                    name=f"osb{hl}_{q0}")
                    if q0 < 3:
                        nc.vector.tensor_add(osb[:], ps2[:], bp_bc[:])
                        dstap = bass.AP(
                            tensor=out,
                            offset=(hl * 1024 + q0) * 256,
                            ap=[[4 * 256, 128], [512 * 256, 2], [1, 256]])
                        nc.sync.dma_start(
                            dstap,
                            osb[:].rearrange("p (h c) -> p h c", h=2))
                    else:
                        # last quarter: per-half add+DMA on two queues so the
                        # final (tail-critical) output transfer is halved
                        for h in range(2):
                            sl = slice(h * 256, (h + 1) * 256)
                            nc.vector.tensor_add(
                                osb[:, sl], ps2[:, sl], bp_bc[:, sl])
                            dstap = bass.AP(
                                tensor=out,
                                offset=(hl * 1024 + q0 + h * 512) * 256,
                                ap=[[4 * 256, 128], [1, 256]])
                            eng = nc.sync if h == 0 else nc.scalar
                            eng.dma_start(dstap, osb[:, sl])

    nc.compile()
    return nc


def _prep_inputs(x, y, q_w, q_gamma, q_beta, q_mean, q_var,
                 kv_w, kv_gamma, kv_beta, kv_mean, kv_var, proj_w, proj_b):
    f = np.float32
    bf = ml_dtypes.bfloat16
    x = np.ascontiguousarray(np.asarray(x, f))
    y = np.ascontiguousarray(np.asarray(y, f))

    gq = np.asarray(q_gamma, f) / np.sqrt(np.asarray(q_var, f) + BN_EPS)
    bq_full = np.asarray(q_beta, f) - np.asarray(q_mean, f) * gq
    wq_host = np.ascontiguousarray((np.asarray(q_w, f) * gq[:, None]).T).astype(bf)

    gkv = np.asarray(kv_gamma, f) / np.sqrt(np.asarray(kv_var, f) + BN_EPS)
    bkv_full = np.asarray(kv_beta, f) - np.asarray(kv_mean, f) * gkv
    wkv_host = np.ascontiguousarray((np.asarray(kv_w, f) * gkv[:, None]).T).astype(bf)

    wp_host = np.ascontiguousarray(np.asarray(proj_w, f).T).astype(bf)
    bp_host = np.asarray(proj_b, f)

    bq2 = np.tile(bq_full[None, :], (1, 2)).astype(bf)
    bkv4_h = np.tile(bkv_full[None, :], (1, 4)).astype(bf)
    bkvr_h = bkv_full[None, :].astype(bf)
    bp2 = np.tile(bp_host[None, :], (1, 2)).astype(f)

    def fold(a):
        # [256, W] -> [128, 2W]: row halves side by side (one 2D DMA)
        return np.ascontiguousarray(
            np.concatenate([a[:128], a[128:]], axis=1))

    biases = np.concatenate([bq2, bkv4_h, bkvr_h], axis=1).astype(bf)
    wq_f = fold(wq_host)
    wkv_f = fold(wkv_host)
    wp_f = fold(wp_host)

    in_maps = []
    for core in range(8):
        b, hp = core // 2, core % 2
        X4 = x[b].reshape(C, N_TOK)
        Y4 = y[b].reshape(C, N_TOK)
        xqa = np.ascontiguousarray(
            X4.reshape(C, 16, 256)[:, :, hp * 128 : (hp + 1) * 128]
        ).reshape(C, 2048).astype(bf)
        Y8 = Y4.reshape(C, 8, 512)
        yka = np.ascontiguousarray(
            Y8[:, :, hp * 128 : (hp + 1) * 128]).reshape(C, 1024).astype(bf)
        yva = np.ascontiguousarray(
            Y8[:, :, 256 + hp * 128 : 256 + (hp + 1) * 128]
        ).reshape(C, 1024).astype(bf)
        in_maps.append({
            "xq": fold(xqa), "yk": fold(yka), "yv": fold(yva),
            "wq": wq_f, "wkv": wkv_f, "wp": wp_f,
            "biases": biases, "bp": bp2,
        })
    return in_maps


def _get_nc():
    if "nc" not in _CACHE:
        _CACHE["nc"] = _build()
    return _CACHE["nc"]


def kernel(x, y, H=64, W=64, q_w=None, q_gamma=None, q_beta=None, q_mean=None,
           q_var=None, kv_w=None, kv_gamma=None, kv_beta=None, kv_mean=None,
           kv_var=None, proj_w=None, proj_b=None, _trace=False):
    from concourse.bass_utils import run_bass_kernel_spmd

    nc = _get_nc()
    in_maps = _prep_inputs(x, y, q_w, q_gamma, q_beta, q_mean, q_var,
                           kv_w, kv_gamma, kv_beta, kv_mean, kv_var,
                           proj_w, proj_b)
    kw = {}
    if _trace:
        kw = {"trace": True, "trace_cores": list(range(8))}
    res = run_bass_kernel_spmd(nc, in_maps, list(range(8)), **kw)
    outa = np.empty((B, N_TOK, C), np.float32)
    for core in range(8):
        b, hp = core // 2, core % 2
        outa[b, hp * 2048 : (hp + 1) * 2048, :] = res.results[core]["out"]
    if _trace:
        return outa, res
    return outa


# revision 45
# speedup vs baseline: 1.0326x; 1.0149x over previous
"""Trainium2 Bass kernel for nn_Attention_57423712748130.

Computation (per batch b):
  X4 = x[b] viewed (C=256, N=4096)   [raw reshape]
  Q4 = silu(BN(q_w @ X4))            (256, 4096)
  KV4 = silu(BN(kv_w @ Y4))          (128, 4096)
  q[n,h,d]  = Q4[n1, n0*256+h*64+d]      n = n1*16+n0
  k[m,h,d]  = KV4[m1, m0*512 + h*64+d]   m = m1*8+m0
  v[m,h,d]  = KV4[m1, m0*512+256+h*64+d]
  att = softmax(q k^T / 8); o = att v
  out rows [h*1024,(h+1)*1024) = O_h @ proj_w.T + proj_b
    where O_h[n2, n3*64+d] = o[4*n2+n3, d]

Sharding: 8 cores = (batch b in 0..3) x (head-pair hp in 0..1); each core
computes heads {2hp, 2hp+1} of batch b = rows [hp*2048,(hp+1)*2048) of out[b].

On-core strategy (v2 — ScalarE(exp)-bound design, ~all else overlapped):
 - all bulk inputs/weights arrive as bf16 host-prefolded to [128, 2W]
   (row halves side by side) and load as 2D piece-DMAs spread over the 3
   DMA queues (sync/scalar/gpsimd; ~2.6us serial latency per DMA/queue,
   ~230 GB/s shared port); output returns bf16, upcast on host
 - conv outputs are computed directly in transposed layout so q^T/k^T
   need no transposes; conv bias is a trailing K=1 matmul (ones (x)
   bias_row) in the same PSUM accumulation; silu computed as
   z*(1+tanh(z/2)) = 2*silu(z) (tanh shares the ACT table set with exp,
   preloaded via a dummy op at t=0); the 2x is folded into the exp scale
   (1/32) and the 2.0 ones-column of V
 - a ~3.4us warmup burst of full-array matmuls on the first-landed
   weight flips the HAM clock gate to 8/8 (2.4 GHz) before real work
 - scoresT[m,n]: ONE [128,1024] psum per m0-block holds BOTH heads side
   by side, so the two score matmuls (PE row groups 0-63/64-127) become
   ready together and overlap in the array; one [128,1024] exp each;
   scores run one t2 AHEAD of exp so ScalarE (the bottleneck: 64 exp +
   12 tanh ops) never waits; av interleaves both heads by m0, chasing
   exp pairwise
 - att@v contracts over m with an extra 2.0-column on V producing
   softmax denominators as psum row 64; 1/denom = one DVE copy to SBUF
   (reciprocal_approx_fast reads garbage from PSUM on HW) + one
   reciprocal_approx_fast, gpsimd partition_broadcast per 256-col half,
   multiply folded into the PSUM-evacuation copy — no DRAM bounce
 - normalization + per-head projection run per q0-quarter, pipelined
   behind the attention of later quarters; O_h columns kept q0-major so
   every PSUM evacuation is contiguous; the output DMA un-permutes rows
"""

import ml_dtypes
import numpy as np

B = 4
N_TOK = 4096
C = 256
BN_EPS = 1e-5

_CACHE = {}


def _build():
    import concourse.bacc as bacc
    import concourse.bass as bass
    import concourse.tile as tile
    from concourse import mybir

    f32 = mybir.dt.float32
    bf16 = mybir.dt.bfloat16
    adt = bf16
    AF = mybir.ActivationFunctionType

    nc = bacc.Bacc("TRN2", target_bir_lowering=False, debug=False, num_devices=8)

    # all [256,W] operands arrive host-prefolded to [128, 2W] (row half
    # c0 side by side) so each loads with ONE 2D DMA; bf16 bias rows are
    # packed into one tensor (per-DMA queue latency is ~2.5-3us, serial)
    xq = nc.dram_tensor("xq", [128, 4096], bf16, kind="ExternalInput")
    yk = nc.dram_tensor("yk", [128, 2048], bf16, kind="ExternalInput")
    yv = nc.dram_tensor("yv", [128, 2048], bf16, kind="ExternalInput")
    wq = nc.dram_tensor("wq", [128, 512], bf16, kind="ExternalInput")
    wkv = nc.dram_tensor("wkv", [128, 256], bf16, kind="ExternalInput")
    wp = nc.dram_tensor("wp", [128, 512], bf16, kind="ExternalInput")
    biases = nc.dram_tensor("biases", [1, 1152], bf16, kind="ExternalInput")
    bp = nc.dram_tensor("bp", [1, 512], f32, kind="ExternalInput")
    out = nc.dram_tensor("out", [2048, 256], bf16, kind="ExternalOutput")

    with tile.TileContext(nc) as tc:
        with (
            tc.tile_pool(name="const", bufs=1) as cp,
            tc.tile_pool(name="actt", bufs=3) as actt,
            tc.tile_pool(name="attp", bufs=24) as attp,
            tc.tile_pool(name="outp", bufs=3) as outp,
            tc.tile_pool(name="nrm", bufs=3) as nrm,
            tc.tile_pool(name="gp", bufs=3) as gp,
            tc.tile_pool(name="psc", bufs=4, space="PSUM") as psc,
            tc.tile_pool(name="pss", bufs=2, space="PSUM") as pss,
        ):
            # ---- load weights / inputs: one 2D DMA each, critical-first
            # per queue (sync/scalar/gpsimd each serialize their DMAs) ----
            def load1(t_dram, shape, tag, eng, dt=bf16, pieces=1):
                t = cp.tile(shape, dt, tag=tag, name=tag)
                w = shape[-1] // pieces
                for p in range(pieces):
                    eng.dma_start(
                        t[:, p * w : (p + 1) * w],
                        t_dram.ap()[:, p * w : (p + 1) * w])
                return t

            # big tensors split into ~128-256KB pieces: each DMA lands on
            # its own HW DMA engine (~55-60 GB/s each), so pieces transfer
            # in parallel even on one queue
            # q_conv(t2) needs xq cols [2t2*128,(2t2+2)*128) of BOTH c0
    
            # halves: first piece-pair rides ScalarE (before the ACT-table
            # preload), the rest ride sync behind wq/yk; biases tiny-first
            # on gpsimd so the trailing conv-bias matmuls never stall
            xqc = cp.tile([128, 4096], bf16, tag="xq", name="xq")
            xq_sb = [xqc[:, i * 2048 : (i + 1) * 2048] for i in range(2)]
            def xq_piece(p, eng):
                for i in range(2):
                    sl = slice(i * 2048 + p * 1024, i * 2048 + (p + 1) * 1024)
                    eng.dma_start(xqc[:, sl], xq.ap()[:, sl])
            xq_piece(0, nc.scalar)
            bias_sb = load1(biases, [1, 1152], "biases", nc.gpsimd)
            wkvc = load1(wkv, [128, 256], "wkv", nc.sync)
            ykc = load1(yk, [128, 2048], "yk", nc.sync, pieces=2)
            yvc = load1(yv, [128, 2048], "yv", nc.gpsimd, pieces=2)
            wpc = load1(wp, [128, 512], "wp", nc.gpsimd)
            bp_bc = cp.tile([128, 512], f32, tag="bp_bc", name="bp_bc")
            nc.gpsimd.dma_start(bp_bc[:], bp.ap().partition_broadcast(128))
            # xq is issued on ScalarE AFTER the ACT-table preload below, so
            # the yk transfers get the port first and the table DMA overlaps
            wkv_sb = [wkvc[:, i * 128 : (i + 1) * 128] for i in range(2)]
            yk_sb = [ykc[:, i * 1024 : (i + 1) * 1024] for i in range(2)]
            yv_sb = [yvc[:, i * 1024 : (i + 1) * 1024] for i in range(2)]
            wp_sb = [wpc[:, i * 256 : (i + 1) * 256] for i in range(2)]
            bq_sb = bias_sb[0:1, 0:512]
            bkv4_sb = bias_sb[0:1, 512:1024]
            bkvr_sb = bias_sb[0:1, 1024:1152]

            ones_row = cp.tile([1, 512], bf16, tag="ones", name="ones")
            nc.vector.memset(ones_row[:], 1.0)
            # preload the exp/tanh ACT table set during the input-DMA wait
            dum_t = actt.tile([128, 512], f32, tag="silu_t", name="dum_t")
            nc.scalar.activation(dum_t[0:1, :], ones_row[:], AF.Tanh, scale=0.5)
            wqc = load1(wq, [128, 512], "wq", nc.scalar)
            xq_piece(1, nc.sync)
            wq_sb = [wqc[:, i * 256 : (i + 1) * 256] for i in range(2)]



            # conv epilogue: psum z already includes bias (K=1 bias matmul).
            # t = tanh(z/2); u = z*t; dst = z + u = z*(1+tanh(z/2)) = 2silu(z)
            def silu_epi(ps, dst_ap, tag, rr=None):
                t = actt.tile([128, 512], f32, tag="silu_t", name=f"t_{tag}")
                u = actt.tile([128, 512], f32, tag="silu_u", name=f"u_{tag}")
                nc.scalar.activation(t[:], ps, AF.Tanh, scale=0.5)
                nc.vector.tensor_mul(u[:], ps, t[:])
                psv, uv = ps, u[:]
                if rr is not None:
                    psv = psv.rearrange(rr, a=4, h=2)
                    uv = uv.rearrange(rr, a=4, h=2)
                nc.vector.tensor_add(dst_ap, psv, uv)

            # ---- PE warmup on wkv (lands first): ~3.4us of full-array
            # matmuls inside the yk-DMA wait window flips the HAM clock
            # gate to 8/8 (2.4 GHz) before the real work starts.
            wps = psc.tile([128, 512], f32, tag="cnv", name="warm")
            for wi in range(12):
                nc.tensor.matmul(
                    wps[:], lhsT=xqc[:, 0:128], rhs=xqc[:, 0:512],
                    start=(wi == 0), stop=(wi == 11))

            # ---- kv conv (k part): kT[pp, m0, m1], pp = hl*64+d ----
            kT = cp.tile([128, 8, 128], adt, tag="kT")
            for mt in range(2):  # m0 quads
                ps = psc.tile([128, 512], f32, tag="cnv", name=f"psk{mt}")
                for mi in range(4):
                    m0 = 4 * mt + mi
                    for c0 in range(2):
                        nc.tensor.matmul(
                            ps[:, mi * 128 : (mi + 1) * 128],
                            lhsT=yk_sb[c0][:, m0 * 128 : (m0 + 1) * 128],
                            rhs=wkv_sb[c0],
                            start=(mi == 0 and c0 == 0), stop=False)
                nc.tensor.matmul(
                    ps[:], lhsT=ones_row[:, 0:128], rhs=bkv4_sb,
                    start=False, stop=True)
                silu_epi(
                    ps[:],
                    kT[:, 4 * mt : 4 * mt + 4, :].rearrange("p a b -> p (a b)"),
                    f"k{mt}")

            # ---- attention state (both heads) ----
            qT = cp.tile([128, 16, 256], adt, tag="qT")
            outun = [
                [cp.tile([128, 1024], adt, tag=f"outun{hl}_{i}",
                         name=f"outun{hl}_{i}") for i in range(2)]
                for hl in range(2)
            ]

            # q conv for one n0-pair
            def q_conv(t2):
                ps = psc.tile([128, 512], f32, tag="cnv", name=f"psq{t2}")
                for nn in range(2):
                    n0 = 2 * t2 + nn
                    for c0 in range(2):
                        nc.tensor.matmul(
                            ps[:, nn * 256 : (nn + 1) * 256],
                            lhsT=xq_sb[c0][:, n0 * 128 : (n0 + 1) * 128],
                            rhs=wq_sb[c0],
                            start=(nn == 0 and c0 == 0), stop=False)
                nc.tensor.matmul(
                    ps[:], lhsT=ones_row[:, 0:128], rhs=bq_sb,
                    start=False, stop=True)
                silu_epi(
                    ps[:],
                    qT[:, 2 * t2 : 2 * t2 + 2, :].rearrange("p a b -> p (a b)"),
                    f"q{t2}")

            q_conv(0)
            q_conv(1)

            # t2-outer attention. One [128,1024] score psum per m0 holds BOTH
            # heads (hl0 cols 0-511, hl1 cols 512-1023): the two score
            # matmuls target PE row-groups 0-63/64-127, become ready
            # together, and issue back-to-back -> they overlap in the array
            # (halving score time). Scores run one t2 AHEAD of exp so
            # ScalarE (the bottleneck) never waits.
            def scores_m0(t2, m0):
                scps = pss.tile([128, 1024], f32, tag="scp",
                                name=f"scp_{t2}_{m0}")
                for hl in range(2):
                    r0, r1 = hl * 64, (hl + 1) * 64
                    nc.tensor.matmul(
                        scps[:, hl * 512 : (hl + 1) * 512],
                        lhsT=kT[r0:r1, m0, :],
                        rhs=qT[r0:r1, 2 * t2 : 2 * t2 + 2, :],
                        start=True, stop=True)
                return scps

            def scores(t2):
                return [scores_m0(t2, m0) for m0 in range(8)]

            # ---- kv conv (v part): vext[m1, m0, hl, 0:64]=2v, [...,64]=2 ----
            vext = cp.tile([128, 8, 2, 65], adt, tag="vext")
            nc.vector.memset(vext[:], 2.0)
            for jv in range(2):
                ps = psc.tile([128, 512], f32, tag="cnv", name=f"psv{jv}")
                for c0 in range(2):
                    nc.tensor.matmul(
                        ps[:], lhsT=wkv_sb[c0],
                        rhs=yv_sb[c0][:, jv * 512 : (jv + 1) * 512],
                        start=(c0 == 0), stop=False)
                nc.tensor.matmul(
                    ps[:], lhsT=bkvr_sb, rhs=ones_row[:],
                    start=False, stop=True)
                silu_epi(
                    ps[:], vext[:, jv * 4 : (jv + 1) * 4, :, 0:64],
                    f"v{jv}", rr="p (a h d) -> p a h d")


            sc_cur = scores(0)
            for t2 in range(8):  # n0 pair (n0 = 2*t2 + nn)
                att = []
                for m0 in range(8):
                    a = attp.tile([128, 1024], adt, tag="att",
                                  name=f"att_{t2}_{m0}")
                    # scoresT = 4*q.k ; want exp(q.k/8) -> scale 1/32
                    nc.scalar.activation(
                        a[:], sc_cur[m0][:], AF.Exp, scale=0.03125)
                    att.append(a)

                # software-pipelined q conv (PE work fills exp-wait)
                if t2 + 2 < 8:
                    q_conv(t2 + 2)

                # av: both heads interleaved by m0 so the PE chases exp
                # pairwise (never more than one att tile behind); the first
                # next-t2 score pairs slot between the last av pairs so
                # exp(t2+1,0) is ready the moment exp(t2,7) retires
                opss = {}
                for hl in range(2):
                    opss[hl] = psc.tile([128, 512], f32, tag="cnv",
                                        name=f"ops{hl}_{t2}")

                def av_pair(m0):
                    for hl in range(2):
                        nc.tensor.matmul(
                            opss[hl][0:65, :], lhsT=vext[:, m0, hl, :],
                            rhs=att[m0][:, hl * 512 : (hl + 1) * 512],
                            start=(m0 == 0), stop=(m0 == 7))


                for m0 in range(6):
                    av_pair(m0)
                if t2 + 1 < 8:
                    nxt = [scores_m0(t2 + 1, 0)]
                    av_pair(6)
                    nxt.append(scores_m0(t2 + 1, 1))
                    av_pair(7)
                    nxt += [scores_m0(t2 + 1, m0) for m0 in range(2, 8)]
                    sc_cur = nxt
                else:
                    av_pair(6)
                    av_pair(7)

                # normalize: 1/denom off psum row 64 (SBUF hop), partition-
                # broadcast, multiply folded into the PSUM evacuation; at odd
                # t2 each head's projection follows its normalize directly so
                # proj(hl0) overlaps normalize(hl1)
                for hl in range(2):
                    ops = opss[hl]
                    # recip_approx_fast is wrong straight off PSUM (HW bit-
                    # trick breaks on the PSUM read path) — copy to SBUF
                    # first; at the last t2 ScalarE is idle, use it instead
                    drow = nrm.tile([1, 512], f32, tag="drow",
                                    name=f"drow{hl}_{t2}")
                    if t2 == 7:
                        nc.scalar.copy(drow[:], ops[64:65, :])
                    else:
                        nc.vector.tensor_copy(drow[:], ops[64:65, :])
                    rrow = nrm.tile([1, 512], f32, tag="rrow",
                                    name=f"rrow{hl}_{t2}")
                    nc.vector.reciprocal_approx_fast(
                        out=rrow[:], in_=drow[:])
                    g = gp.tile([64, 512], f32, tag="g", name=f"g{hl}_{t2}")
                    c0 = t2 % 2
                    q0 = t2 // 2
                    for nn in range(2):
                        # broadcast + evacuate per 256-col half so the first
                        # multiply starts while the second half broadcasts
                        sl = slice(nn * 256, (nn + 1) * 256)
                        nc.gpsimd.partition_broadcast(
                            g[0:64, sl], rrow[0:1, sl], channels=64)
                        # o * (1/denom) -> outun[c][band, q0-major cols]
                        dst = outun[hl][c0][
                            nn * 64 : nn * 64 + 64,
                            q0 * 256 : (q0 + 1) * 256]
                        nc.vector.tensor_mul(
                            dst,
                            ops[0:64, nn * 256 : (nn + 1) * 256],
                            g[0:64, sl])

                    if t2 % 2 == 0:
                        continue
                    # ---- quarter q0 complete for this head: projection ----
                    # proj fc in {2q0, 2q0+1}; rows hl*1024+half*512+q0+4r
                    ps2 = psc.tile([128, 512], f32, tag="cnv",
                                   name=f"psproj{hl}_{q0}")
                    for half in range(2):
                        fc = 2 * q0 + half
                        for cc in range(2):
                            nc.tensor.matmul(
                                ps2[:, half * 256 : (half + 1) * 256],
                                lhsT=outun[hl][cc][
                                    :, fc * 128 : (fc + 1) * 128],
                                rhs=wp_sb[cc],
                                start=(cc == 0), stop=(cc == 1))
                    osb = outp.tile([128, 512], bf16, tag="osb",
                                    name=f"osb{hl}_{q0}")
                    if q0 < 3:
                        nc.vector.tensor_add(osb[:], ps2[:], bp_bc[:])
                        dstap = bass.AP(
                            tensor=out,
                            offset=(hl * 1024 + q0) * 256,
                            ap=[[4 * 256, 128], [512 * 256, 2], [1, 256]])
                        nc.sync.dma_start(
                            dstap,
                            osb[:].rearrange("p (h c) -> p h c", h=2))
                    else:
                        # last quarter: per-half add+DMA on two queues so the
                        # final (tail-critical) output transfer is halved
                        for h in range(2):
                            sl = slice(h * 256, (h + 1) * 256)
                            nc.vector.tensor_add(
                                osb[:, sl], ps2[:, sl], bp_bc[:, sl])
                            dstap = bass.AP(
                                tensor=out,
                                offset=(hl * 1024 + q0 + h * 512) * 256,
                                ap=[[4 * 256, 128], [1, 256]])
                            eng = nc.sync if h == 0 else nc.scalar
                            eng.dma_start(dstap, osb[:, sl])

    nc.compile()
    return nc


def _prep_inputs(x, y, q_w, q_gamma, q_beta, q_mean, q_var,
                 kv_w, kv_gamma, kv_beta, kv_mean, kv_var, proj_w, proj_b):
    f = np.float32
    bf = ml_dtypes.bfloat16
    x = np.ascontiguousarray(np.asarray(x, f))
    y = np.ascontiguousarray(np.asarray(y, f))

    gq = np.asarray(q_gamma, f) / np.sqrt(np.asarray(q_var, f) + BN_EPS)
    bq_full = np.asarray(q_beta, f) - np.asarray(q_mean, f) * gq
    wq_host = np.ascontiguousarray((np.asarray(q_w, f) * gq[:, None]).T).astype(bf)

    gkv = np.asarray(kv_gamma, f) / np.sqrt(np.asarray(kv_var, f) + BN_EPS)
    bkv_full = np.asarray(kv_beta, f) - np.asarray(kv_mean, f) * gkv
    wkv_host = np.ascontiguousarray((np.asarray(kv_w, f) * gkv[:, None]).T).astype(bf)

    wp_host = np.ascontiguousarray(np.asarray(proj_w, f).T).astype(bf)
    bp_host = np.asarray(proj_b, f)

    bq2 = np.tile(bq_full[None, :], (1, 2)).astype(bf)
    bkv4_h = np.tile(bkv_full[None, :], (1, 4)).astype(bf)
    bkvr_h = bkv_full[None, :].astype(bf)
    bp2 = np.tile(bp_host[None, :], (1, 2)).astype(f)

    def fold(a):
        # [256, W] -> [128, 2W]: row halves side by side (one 2D DMA)
        return np.ascontiguousarray(
            np.concatenate([a[:128], a[128:]], axis=1))

    biases = np.concatenate([bq2, bkv4_h, bkvr_h], axis=1).astype(bf)
    wq_f = fold(wq_host)
    wkv_f = fold(wkv_host)
    wp_f = fold(wp_host)

    in_maps = []
    for core in range(8):
        b, hp = core // 2, core % 2
        X4 = x[b].reshape(C, N_TOK)
        Y4 = y[b].reshape(C, N_TOK)
        xqa = np.ascontiguousarray(
            X4.reshape(C, 16, 256)[:, :, hp * 128 : (hp + 1) * 128]
        ).reshape(C, 2048).astype(bf)
        Y8 = Y4.reshape(C, 8, 512)
        yka = np.ascontiguousarray(
            Y8[:, :, hp * 128 : (hp + 1) * 128]).reshape(C, 1024).astype(bf)
        yva = np.ascontiguousarray(
            Y8[:, :, 256 + hp * 128 : 256 + (hp + 1) * 128]
        ).reshape(C, 1024).astype(bf)
        in_maps.append({
            "xq": fold(xqa), "yk": fold(yka), "yv": fold(yva),
            "wq": wq_f, "wkv": wkv_f, "wp": wp_f,
            "biases": biases, "bp": bp2,
        })
    return in_maps


def _get_nc():
    if "nc" not in _CACHE:
        _CACHE["nc"] = _build()
    return _CACHE["nc"]


def kernel(x, y, H=64, W=64, q_w=None, q_gamma=None, q_beta=None, q_mean=None,
           q_var=None, kv_w=None, kv_gamma=None, kv_beta=None, kv_mean=None,
           kv_var=None, proj_w=None, proj_b=None, _trace=False):
    from concourse.bass_utils import run_bass_kernel_spmd

    nc = _get_nc()
    in_maps = _prep_inputs(x, y, q_w, q_gamma, q_beta, q_mean, q_var,
                           kv_w, kv_gamma, kv_beta, kv_mean, kv_var,
                           proj_w, proj_b)
    kw = {}
    if _trace:
        kw = {"trace": True, "trace_cores": list(range(8))}
    res = run_bass_kernel_spmd(nc, in_maps, list(range(8)), **kw)
    outa = np.empty((B, N_TOK, C), np.float32)
    for core in range(8):
        b, hp = core // 2, core % 2
        outa[b, hp * 2048 : (hp + 1) * 2048, :] = res.results[core]["out"]
    if _trace:
        return outa, res
    return outa
